# revision 1
# baseline (speedup 1.0000x reference)
"""GAT (2-head, concat=False) over B*T=8 timesteps, one timestep per NeuronCore.

Per core (timestep t):
  Phase 1: project xt @ Wplus on PE -> gather table T[n] = [xl_h0|1|0|xl_h1|1|0|as0|as1]
           (fp16, 134 cols) + adT[n] = [ad0, ad1] (fp16), written to DRAM.
  Phase 2: edges sorted by dst, tiled 128-dst-node tiles, 128-edge subtiles.
           Batched indirect-DMA gathers rows T[src[e]] and adT[dst[e]];
           scores s = as+ad -> leaky_relu -> exp on ACT; per-edge-tile selection
           matrix SE[e,n] = (iota[n]==dstrel[e]) via one tensor_scalar; messages
           msg = G * ex via per-partition-scalar tensor_scalar; segment-sum via
           PE matmul accumulating PSUM [128 nodes, 132]; finalize = head-mean /
           denom + bias + ELU, DMA out.

Host side: edge sorting/padding/layout + output reassembly only (no FLOPs on h).
"""
import math
import numpy as np
from contextlib import ExitStack

import concourse.bass as bass
import concourse.tile as tile
from concourse import mybir
from concourse.bass_utils import run_bass_kernel_spmd

P = 128
NEG_SLOPE = 0.2
# Dummy rows score as = -40 -> ex = exp(-8) = 3.4e-4: representable in fp16
# (so pad-node denominators stay finite) yet only reaches pad-node outputs,
# which the host trims; real nodes see SE=0 for dummy edges.
DUMMY_AS = -40.0


def _prep_graph(src, dst, n_nodes, n_pad):
    """Sort/pad edges into (128-dst-node tile, 128-edge subtile) layout.

    Returns dict with per-tile subtile counts and [128, KTOT] index planes.
    """
    src = np.asarray(src).astype(np.int64)
    dst = np.asarray(dst).astype(np.int64)
    # dummy self-referencing edges for pad nodes so every dst row has denom > 0
    if n_pad > n_nodes:
        pad_dst = np.arange(n_nodes, n_pad, dtype=np.int64)
        src = np.concatenate([src, np.full(len(pad_dst), n_pad, dtype=np.int64)])
        dst = np.concatenate([dst, pad_dst])
    order = np.argsort(dst, kind="stable")
    src_s, dst_s = src[order], dst[order]
    nt_count = n_pad // P
    # edge range per node tile
    bounds = np.searchsorted(dst_s, np.arange(0, n_pad + P, P))
    g_per_tile = []
    src_cols, dst_cols, drel_cols = [], [], []
    for nt in range(nt_count):
        lo, hi = bounds[nt], bounds[nt + 1]
        cnt = hi - lo
        g = max(1, math.ceil(cnt / P))
        g_per_tile.append(g)
        s = np.full(g * P, n_pad, dtype=np.int64)   # dummy row index
        d = np.full(g * P, n_pad, dtype=np.int64)   # adT dummy row (zeros)
        r = np.full(g * P, 255.0, dtype=np.float32)  # no iota match
        s[:cnt] = src_s[lo:hi]
        d[:cnt] = dst_s[lo:hi]
        r[:cnt] = (dst_s[lo:hi] - nt * P).astype(np.float32)
        # edge e_local -> partition e_local % 128, subtile e_local // 128
        src_cols.append(s.reshape(g, P).T)
        dst_cols.append(d.reshape(g, P).T)
        drel_cols.append(r.reshape(g, P).T)
    return {
        "g_per_tile": g_per_tile,
        "src_plane": np.concatenate(src_cols, axis=1).astype(np.int32),
        "dst_plane": np.concatenate(dst_cols, axis=1).astype(np.int32),
        "drel_plane": np.concatenate(drel_cols, axis=1).astype(np.float32),
    }


def _build_nc(n_pad, g_per_tile, q_tiles, legalize=True):
    """Build the Bass module. n_pad: padded node count (mult of 128)."""
    f16, f32, i32 = mybir.dt.float16, mybir.dt.float32, mybir.dt.int32
    nt_count = n_pad // P
    npp = n_pad + P  # +dummy block
    ktot = sum(g_per_tile)

    nc = bass.Bass()
    XTT = nc.dram_tensor("XTT", [65, n_pad], f32, kind="ExternalInput")
    WPLUS = nc.dram_tensor("WPLUS", [65, 136], f32, kind="ExternalInput")
    IOTA = nc.dram_tensor("IOTA", [P, P], f16, kind="ExternalInput")
    BIASR = nc.dram_tensor("BIASR", [P, 64], f32, kind="ExternalInput")
    DCONST = nc.dram_tensor("DCONST", [P, 136], f16, kind="ExternalInput")
    SRCI = nc.dram_tensor("SRCI", [P, ktot], i32, kind="ExternalInput")
    DSTI = nc.dram_tensor("DSTI", [P, ktot], i32, kind="ExternalInput")
    DREL = nc.dram_tensor("DREL", [P, ktot], f32, kind="ExternalInput")
    T = nc.dram_tensor("T", [npp, 134], f16, kind="Internal")
    ADT = nc.dram_tensor("ADT", [npp, 2], f16, kind="Internal")
    OUT = nc.dram_tensor("OUT", [n_pad, 64], f32, kind="ExternalOutput")

    with tile.TileContext(nc) as tc:
        with ExitStack() as ctx:
            cpool = ctx.enter_context(tc.tile_pool(name="consts", bufs=1))
            wplus_sb = cpool.tile([65, 136], f32)
            nc.gpsimd.dma_start(wplus_sb[:], WPLUS[:, :])
            iota_sb = cpool.tile([P, P], f16)
            nc.sync.dma_start(iota_sb[:], IOTA[:, :])
            biasr_sb = cpool.tile([P, 64], f32)
            nc.sync.dma_start(biasr_sb[:], BIASR[:, :])

            # ---------------- Phase 1: build tables ----------------
            p1 = ctx.enter_context(tc.tile_pool(name="p1", bufs=3))
            p1ps = ctx.enter_context(tc.tile_pool(name="p1ps", bufs=2, space="PSUM"))
            for c in range(nt_count):
                xtt_sb = p1.tile([65, P], f32, tag="xtt")
                nc.gpsimd.dma_start(xtt_sb[:], XTT[:, c * P:(c + 1) * P])
                ps = p1ps.tile([P, 136], f32, tag="ps")
                # ones-row of XTT x WPLUS row 64 supplies the constant-1 cols
                nc.tensor.matmul(ps[:], lhsT=xtt_sb[:], rhs=wplus_sb[:],
                                 start=True, stop=True)
                tsb = p1.tile([P, 134], f16, tag="tsb")
                nc.scalar.copy(tsb[:], ps[:, 0:134])
                adsb = p1.tile([P, 2], f16, tag="adsb")
                nc.vector.tensor_copy(adsb[:], ps[:, 134:136])
                nc.sync.dma_start(T[c * P:(c + 1) * P, :], tsb[:])
                nc.sync.dma_start(ADT[c * P:(c + 1) * P, :], adsb[:])
            dsb = p1.tile([P, 136], f16, tag="dsb")
            nc.sync.dma_start(dsb[:], DCONST[:, :])
            nc.sync.dma_start(T[n_pad:npp, :], dsb[:, 0:134])
            nc.sync.dma_start(ADT[n_pad:npp, :], dsb[:, 134:136])

            # ---------------- Phase 2: edges ----------------
            gp = ctx.enter_context(tc.tile_pool(name="gp", bufs=2))
            sep = ctx.enter_context(tc.tile_pool(name="sep", bufs=4))
            msp = ctx.enter_context(tc.tile_pool(name="msp", bufs=4))
            scp = ctx.enter_context(tc.tile_pool(name="scp", bufs=3))
            fin = ctx.enter_context(tc.tile_pool(name="fin", bufs=3))
            accp = ctx.enter_context(tc.tile_pool(name="accp", bufs=4, space="PSUM"))

            batches = []
            nt = 0
            while nt < nt_count:
                hi = min(nt + q_tiles, nt_count)
                batches.append((nt, hi))
                nt = hi
            col0 = 0
            for (lo, hi) in batches:
                kq = sum(g_per_tile[lo:hi])
                srci = scp.tile([P, kq], i32, tag="srci")
                nc.sync.dma_start(srci[:], SRCI[:, col0:col0 + kq])
                dsti = scp.tile([P, kq], i32, tag="dsti")
                nc.sync.dma_start(dsti[:], DSTI[:, col0:col0 + kq])
                drel = scp.tile([P, kq], f32, tag="drel")
                nc.sync.dma_start(drel[:], DREL[:, col0:col0 + kq])

                # HW DynamicAP applies ONE indirect offset per partition
                # (first index) and streams the rest contiguously, so each
                # gather instruction must use a [P,1] offset column.
                CH = 1
                gq = gp.tile([P, kq * 134], f16, tag="gq")
                for c0 in range(0, kq, CH):
                    c1 = min(c0 + CH, kq)
                    nc.gpsimd.indirect_dma_start(
                        out=gq[:, c0 * 134:c1 * 134], out_offset=None, in_=T[:, :],
                        in_offset=bass.IndirectOffsetOnAxis(ap=srci[:, c0:c1], axis=0))
                adq = gp.tile([P, kq * 2], f16, tag="adq")
                for c0 in range(0, kq, CH):
                    c1 = min(c0 + CH, kq)
                    nc.gpsimd.indirect_dma_start(
                        out=adq[:, c0 * 2:c1 * 2], out_offset=None, in_=ADT[:, :],
                        in_offset=bass.IndirectOffsetOnAxis(ap=dsti[:, c0:c1], axis=0))

                # scores for the whole batch: s = as + ad ; lr ; ex
                gq3 = gq[:].rearrange("p (k c) -> p k c", c=134)
                s_t = scp.tile([P, kq * 2], f32, tag="s_t")
                s3 = s_t[:].rearrange("p (k h) -> p k h", h=2)
                nc.vector.tensor_tensor(out=s3, in0=gq3[:, :, 132:134],
                                        in1=adq[:].rearrange("p (k h) -> p k h", h=2),
                                        op=mybir.AluOpType.add)
                lr_t = scp.tile([P, kq * 2], f32, tag="lr_t")
                nc.vector.scalar_tensor_tensor(
                    out=lr_t[:], in0=s_t[:], scalar=NEG_SLOPE, in1=s_t[:],
                    op0=mybir.AluOpType.mult, op1=mybir.AluOpType.max)
                ex_t = scp.tile([P, kq * 2], f32, tag="ex_t")
                nc.scalar.activation(ex_t[:], lr_t[:],
                                     mybir.ActivationFunctionType.Exp)

                j = 0
                for t_i in range(lo, hi):
                    g_n = g_per_tile[t_i]
                    acc = accp.tile([P, 132], f32, tag="acc")
                    for g in range(g_n):
                        se = sep.tile([P, P], f16, tag="se")
                        nc.vector.tensor_scalar(
                            out=se[:], in0=iota_sb[:],
                            scalar1=drel[:, j:j + 1], scalar2=None,
                            op0=mybir.AluOpType.is_equal)
                        msg = msp.tile([P, 132], f16, tag="msg")
                        nc.vector.tensor_scalar(
                            out=msg[:, 0:66], in0=gq[:, j * 134:j * 134 + 66],
                            scalar1=ex_t[:, 2 * j:2 * j + 1], scalar2=None,
                            op0=mybir.AluOpType.mult)
                        nc.vector.tensor_scalar(
                            out=msg[:, 66:132], in0=gq[:, j * 134 + 66:j * 134 + 132],
                            scalar1=ex_t[:, 2 * j + 1:2 * j + 2], scalar2=None,
                            op0=mybir.AluOpType.mult)
                        nc.tensor.matmul(acc[:], lhsT=se[:], rhs=msg[:],
                                         start=(g == 0), stop=(g == g_n - 1))
                        j += 1
                    # finalize node tile
                    r = fin.tile([P, 2], f32, tag="r")
                    dpair = acc[:].rearrange("p (a b) -> p a b", b=66)[:, :, 64:65]
                    nc.vector.reciprocal(r[:], dpair)
                    t0 = fin.tile([P, 64], f32, tag="t0")
                    nc.vector.tensor_scalar(out=t0[:], in0=acc[:, 0:64],
                                            scalar1=r[:, 0:1], scalar2=None,
                                            op0=mybir.AluOpType.mult)
                    tsum = fin.tile([P, 64], f32, tag="tsum")
                    nc.vector.scalar_tensor_tensor(
                        out=tsum[:], in0=acc[:, 66:130], scalar=r[:, 1:2],
                        in1=t0[:], op0=mybir.AluOpType.mult,
                        op1=mybir.AluOpType.add)
                    b2 = fin.tile([P, 64], f32, tag="b2")
                    nc.vector.tensor_tensor(out=b2[:], in0=tsum[:], in1=biasr_sb[:],
                                            op=mybir.AluOpType.add)
                    q = fin.tile([P, 64], f32, tag="q")
                    nc.scalar.activation(q[:], b2[:],
                                         mybir.ActivationFunctionType.Relu)
                    mn = fin.tile([P, 64], f32, tag="mn")
                    nc.vector.tensor_tensor(out=mn[:], in0=b2[:], in1=q[:],
                                            op=mybir.AluOpType.subtract)
                    e = fin.tile([P, 64], f32, tag="e")
                    nc.scalar.activation(e[:], mn[:],
                                         mybir.ActivationFunctionType.Exp)
                    o = fin.tile([P, 64], f32, tag="o")
                    nc.vector.scalar_tensor_tensor(
                        out=o[:], in0=e[:], scalar=-1.0, in1=q[:],
                        op0=mybir.AluOpType.add, op1=mybir.AluOpType.add)
                    nc.sync.dma_start(OUT[t_i * P:(t_i + 1) * P, :], o[:])
                col0 += kq
    if legalize:
        _split_multi_waits(nc)
    return nc


def _split_multi_waits(nc, max_waits=1):
    """Walrus codegen encodes at most one sync wait on most ISA structs and
    does not reliably spill extras; hoist surplus waits onto same-engine NoOps
    placed immediately before the instruction (engine streams are per-engine
    in-order, so semantics are preserved)."""
    n = 0
    for f in nc.m.functions:
        for b in f.blocks:
            new = []
            for inst in b.instructions:
                si = inst.sync_info
                waits = list(si.on_wait) if si is not None and si.on_wait else []
                if len(waits) > max_waits:
                    extras, keep = waits[:-max_waits], waits[-max_waits:]
                    for w in extras:
                        n += 1
                        new.append(mybir.InstNoOp(
                            name=f"waitnop-{n}",
                            engine=inst.engine,
                            sync_info=mybir.SyncInfo(on_wait=[w], on_update=[]),
                        ))
                    si.on_wait = keep
                new.append(inst)
            b.instructions = new
    return n


def _host_consts(W, att_src, att_dst, bias):
    Fin, H, C = W.shape
    wp = np.zeros((65, 136), np.float32)
    # xl stored pre-scaled by 0.5 (head-mean factor); denom cols unaffected,
    # so msg/denom comes out as 0.5 * sum(alpha*xl) directly.
    wp[:64, 0:64] = 0.5 * W[:, 0, :]
    wp[:64, 66:130] = 0.5 * W[:, 1, :]
    wp[:64, 132] = W[:, 0, :] @ att_src[0]
    wp[:64, 133] = W[:, 1, :] @ att_src[1]
    wp[:64, 134] = W[:, 0, :] @ att_dst[0]
    wp[:64, 135] = W[:, 1, :] @ att_dst[1]
    # ones-row -> constant 1.0 in the denominator columns of every table row
    wp[64, 64] = 1.0
    wp[64, 130] = 1.0
    iota = np.tile(np.arange(P, dtype=np.float16)[None, :], (P, 1))
    biasr = np.tile(np.asarray(bias, np.float32)[None, :], (P, 1))
    dconst = np.zeros((P, 136), np.float16)
    dconst[:, 64] = 1.0
    dconst[:, 130] = 1.0
    dconst[:, 132] = DUMMY_AS
    dconst[:, 133] = DUMMY_AS
    return wp, iota, biasr, dconst


def gat_run(h, src, dst, W, att_src, att_dst, bias, q_tiles=4, run_kwargs=None):
    """Shared implementation: returns (output [B,N,T,C] f32, BassKernelResults)."""
    h = np.asarray(h, np.float32)
    Bb, N, Tt, Fin = h.shape
    n_pad = math.ceil(N / P) * P
    prep = _prep_graph(np.asarray(src), np.asarray(dst), N, n_pad)
    nc = _build_nc(n_pad, prep["g_per_tile"], q_tiles)
    wp, iota, biasr, dconst = _host_consts(
        np.asarray(W, np.float32), np.asarray(att_src, np.float32),
        np.asarray(att_dst, np.float32), np.asarray(bias, np.float32))

    n_cores = Bb * Tt
    in_maps = []
    for t in range(n_cores):
        xtt = np.zeros((65, n_pad), np.float32)
        xtt[:64, :N] = h[0, :, t, :].T
        xtt[64, :] = 1.0
        in_maps.append({
            "XTT": np.ascontiguousarray(xtt),
            "WPLUS": wp, "IOTA": iota, "BIASR": biasr, "DCONST": dconst,
            "SRCI": prep["src_plane"], "DSTI": prep["dst_plane"],
            "DREL": prep["drel_plane"],
        })
    res = run_bass_kernel_spmd(nc, in_maps, core_ids=list(range(n_cores)),
                               **(run_kwargs or {}))
    outs = [res.results[t]["OUT"][:N] for t in range(n_cores)]
    out = np.stack(outs, axis=0).reshape(Bb, Tt, N, 64).transpose(0, 2, 1, 3)
    return np.ascontiguousarray(out.astype(np.float32)), res


def kernel(h, src, dst, W, att_src, att_dst, bias):
    out, _ = gat_run(h, src, dst, W, att_src, att_dst, bias)
    return out



# revision 18
# speedup vs baseline: 1.1616x; 1.1616x over previous
"""GAT (2-head, concat=False) over B*T=8 timesteps, one timestep per NeuronCore.

Per core (timestep t):
  Phase 1: project xt @ Wplus on PE -> gather table T[n] = [xl_h0|1|0|xl_h1|1|0|as0|as1]
           (fp16, 134 cols) + adT[n] = [ad0, ad1] (fp16), written to DRAM.
  Phase 2: edges sorted by dst, tiled 128-dst-node tiles, 128-edge subtiles.
           Batched indirect-DMA gathers rows T[src[e]] and adT[dst[e]];
           scores s = as+ad -> leaky_relu -> exp on ACT; per-edge-tile selection
           matrix SE[e,n] = (iota[n]==dstrel[e]) via one tensor_scalar; messages
           msg = G * ex via per-partition-scalar tensor_scalar; segment-sum via
           PE matmul accumulating PSUM [128 nodes, 132]; finalize = head-mean /
           denom + bias + ELU, DMA out.

Host side: edge sorting/padding/layout + output reassembly only (no FLOPs on h).
"""
import math
import numpy as np
from contextlib import ExitStack

import concourse.bass as bass
import concourse.tile as tile
from concourse import mybir
from concourse.bass_utils import run_bass_kernel_spmd

P = 128
NEG_SLOPE = 0.2
# Dummy rows score as = -40 -> ex = exp(-8) = 3.4e-4: representable in fp16
# (so pad-node denominators stay finite) yet only reaches pad-node outputs,
# which the host trims; real nodes see SE=0 for dummy edges.
DUMMY_AS = -40.0


def _prep_graph(src, dst, n_nodes, n_pad):
    """Sort/pad edges into (128-dst-node tile, 128-edge subtile) layout.

    Returns dict with per-tile subtile counts and [128, KTOT] index planes.
    """
    src = np.asarray(src).astype(np.int64)
    dst = np.asarray(dst).astype(np.int64)
    # dummy self-referencing edges for pad nodes so every dst row has denom > 0
    if n_pad > n_nodes:
        pad_dst = np.arange(n_nodes, n_pad, dtype=np.int64)
        src = np.concatenate([src, np.full(len(pad_dst), n_pad, dtype=np.int64)])
        dst = np.concatenate([dst, pad_dst])
    order = np.argsort(dst, kind="stable")
    src_s, dst_s = src[order], dst[order]
    nt_count = n_pad // P
    # edge range per node tile
    bounds = np.searchsorted(dst_s, np.arange(0, n_pad + P, P))
    g_per_tile = []
    src_cols, dst_cols, drel_cols = [], [], []
    for nt in range(nt_count):
        lo, hi = bounds[nt], bounds[nt + 1]
        cnt = hi - lo
        g = max(1, math.ceil(cnt / P))
        g_per_tile.append(g)
        s = np.full(g * P, n_pad, dtype=np.int64)   # dummy row index
        d = np.full(g * P, n_pad, dtype=np.int64)   # adT dummy row (zeros)
        r = np.full(g * P, 255.0, dtype=np.float32)  # no iota match
        s[:cnt] = src_s[lo:hi]
        d[:cnt] = dst_s[lo:hi]
        r[:cnt] = (dst_s[lo:hi] - nt * P).astype(np.float32)
        # edge e_local -> partition e_local % 128, subtile e_local // 128
        src_cols.append(s.reshape(g, P).T)
        dst_cols.append(d.reshape(g, P).T)
        drel_cols.append(r.reshape(g, P).T)
    return {
        "g_per_tile": g_per_tile,
        "src_plane": np.concatenate(src_cols, axis=1).astype(np.int32),
        "dst_plane": np.concatenate(dst_cols, axis=1).astype(np.int32),
        "drel_plane": np.concatenate(drel_cols, axis=1).astype(np.float32),
    }


def _build_nc(n_pad, g_per_tile, q_tiles, legalize=True):
    """Build the Bass module. n_pad: padded node count (mult of 128)."""
    f16, f32, i32 = mybir.dt.float16, mybir.dt.float32, mybir.dt.int32
    nt_count = n_pad // P
    npp = n_pad + P  # +dummy block
    ktot = sum(g_per_tile)

    nc = bass.Bass()
    XTT = nc.dram_tensor("XTT", [65, n_pad], f32, kind="ExternalInput")
    WPLUS = nc.dram_tensor("WPLUS", [65, 136], f32, kind="ExternalInput")
    IOTA = nc.dram_tensor("IOTA", [P, P], f16, kind="ExternalInput")
    BIASR = nc.dram_tensor("BIASR", [P, 64], f32, kind="ExternalInput")
    DCONST = nc.dram_tensor("DCONST", [P, 136], f16, kind="ExternalInput")
    SRCI = nc.dram_tensor("SRCI", [P, ktot], i32, kind="ExternalInput")
    DSTI = nc.dram_tensor("DSTI", [P, ktot], i32, kind="ExternalInput")
    DREL = nc.dram_tensor("DREL", [P, ktot], f32, kind="ExternalInput")
    T = nc.dram_tensor("T", [npp, 134], f16, kind="Internal")
    ADT = nc.dram_tensor("ADT", [npp, 2], f16, kind="Internal")
    OUT = nc.dram_tensor("OUT", [n_pad, 64], f32, kind="ExternalOutput")

    with tile.TileContext(nc) as tc:
        with ExitStack() as ctx:
            cpool = ctx.enter_context(tc.tile_pool(name="consts", bufs=1))
            wplus_sb = cpool.tile([65, 136], f32)
            nc.gpsimd.dma_start(wplus_sb[:], WPLUS[:, :])
            iota_sb = cpool.tile([P, P], f16)
            nc.sync.dma_start(iota_sb[:], IOTA[:, :])
            biasr_sb = cpool.tile([P, 64], f32)
            nc.sync.dma_start(biasr_sb[:], BIASR[:, :])

            # ---------------- Phase 1: build tables ----------------
            p1 = ctx.enter_context(tc.tile_pool(name="p1", bufs=3))
            p1ps = ctx.enter_context(tc.tile_pool(name="p1ps", bufs=2, space="PSUM"))
            for c in range(nt_count):
                xtt_sb = p1.tile([65, P], f32, tag="xtt")
                nc.gpsimd.dma_start(xtt_sb[:], XTT[:, c * P:(c + 1) * P])
                ps = p1ps.tile([P, 136], f32, tag="ps")
                # ones-row of XTT x WPLUS row 64 supplies the constant-1 cols
                nc.tensor.matmul(ps[:], lhsT=xtt_sb[:], rhs=wplus_sb[:],
                                 start=True, stop=True)
                tsb = p1.tile([P, 134], f16, tag="tsb")
                nc.scalar.copy(tsb[:], ps[:, 0:134])
                adsb = p1.tile([P, 2], f16, tag="adsb")
                nc.vector.tensor_copy(adsb[:], ps[:, 134:136])
                nc.sync.dma_start(T[c * P:(c + 1) * P, :], tsb[:])
                nc.sync.dma_start(ADT[c * P:(c + 1) * P, :], adsb[:])
            dsb = p1.tile([P, 136], f16, tag="dsb")
            nc.sync.dma_start(dsb[:], DCONST[:, :])
            nc.sync.dma_start(T[n_pad:npp, :], dsb[:, 0:134])
            nc.sync.dma_start(ADT[n_pad:npp, :], dsb[:, 134:136])

            # ---------------- Phase 2: edges ----------------
            gp = ctx.enter_context(tc.tile_pool(name="gp", bufs=2))
            sep = ctx.enter_context(tc.tile_pool(name="sep", bufs=4))
            msp = ctx.enter_context(tc.tile_pool(name="msp", bufs=4))
            scp = ctx.enter_context(tc.tile_pool(name="scp", bufs=3))
            fin = ctx.enter_context(tc.tile_pool(name="fin", bufs=3))
            accp = ctx.enter_context(tc.tile_pool(name="accp", bufs=4, space="PSUM"))

            batches = []
            nt = 0
            while nt < nt_count:
                hi = min(nt + q_tiles, nt_count)
                batches.append((nt, hi))
                nt = hi
            col0 = 0
            for (lo, hi) in batches:
                kq = sum(g_per_tile[lo:hi])
                srci = scp.tile([P, kq], i32, tag="srci")
                nc.sync.dma_start(srci[:], SRCI[:, col0:col0 + kq])
                dsti = scp.tile([P, kq], i32, tag="dsti")
                nc.sync.dma_start(dsti[:], DSTI[:, col0:col0 + kq])
                drel = scp.tile([P, kq], f32, tag="drel")
                nc.sync.dma_start(drel[:], DREL[:, col0:col0 + kq])

                # HW DynamicAP applies ONE indirect offset per partition
                # (first index) and streams the rest contiguously, so each
                # gather instruction must use a [P,1] offset column.
                CH = 1
                gq = gp.tile([P, kq * 134], f16, tag="gq")
                for c0 in range(0, kq, CH):
                    c1 = min(c0 + CH, kq)
                    nc.gpsimd.indirect_dma_start(
                        out=gq[:, c0 * 134:c1 * 134], out_offset=None, in_=T[:, :],
                        in_offset=bass.IndirectOffsetOnAxis(ap=srci[:, c0:c1], axis=0))
                adq = gp.tile([P, kq * 2], f16, tag="adq")
                for c0 in range(0, kq, CH):
                    c1 = min(c0 + CH, kq)
                    nc.gpsimd.indirect_dma_start(
                        out=adq[:, c0 * 2:c1 * 2], out_offset=None, in_=ADT[:, :],
                        in_offset=bass.IndirectOffsetOnAxis(ap=dsti[:, c0:c1], axis=0))

                # scores for the whole batch: s = as + ad ; lr ; ex
                gq3 = gq[:].rearrange("p (k c) -> p k c", c=134)
                s_t = scp.tile([P, kq * 2], f32, tag="s_t")
                s3 = s_t[:].rearrange("p (k h) -> p k h", h=2)
                nc.vector.tensor_tensor(out=s3, in0=gq3[:, :, 132:134],
                                        in1=adq[:].rearrange("p (k h) -> p k h", h=2),
                                        op=mybir.AluOpType.add)
                lr_t = scp.tile([P, kq * 2], f32, tag="lr_t")
                nc.vector.scalar_tensor_tensor(
                    out=lr_t[:], in0=s_t[:], scalar=NEG_SLOPE, in1=s_t[:],
                    op0=mybir.AluOpType.mult, op1=mybir.AluOpType.max)
                ex_t = scp.tile([P, kq * 2], f32, tag="ex_t")
                nc.scalar.activation(ex_t[:], lr_t[:],
                                     mybir.ActivationFunctionType.Exp)

                j = 0
                for t_i in range(lo, hi):
                    g_n = g_per_tile[t_i]
                    acc = accp.tile([P, 132], f32, tag="acc")
                    for g in range(g_n):
                        se = sep.tile([P, P], f16, tag="se")
                        nc.vector.tensor_scalar(
                            out=se[:], in0=iota_sb[:],
                            scalar1=drel[:, j:j + 1], scalar2=None,
                            op0=mybir.AluOpType.is_equal)
                        msg = msp.tile([P, 132], f16, tag="msg")
                        nc.vector.tensor_scalar(
                            out=msg[:, 0:66], in0=gq[:, j * 134:j * 134 + 66],
                            scalar1=ex_t[:, 2 * j:2 * j + 1], scalar2=None,
                            op0=mybir.AluOpType.mult)
                        nc.vector.tensor_scalar(
                            out=msg[:, 66:132], in0=gq[:, j * 134 + 66:j * 134 + 132],
                            scalar1=ex_t[:, 2 * j + 1:2 * j + 2], scalar2=None,
                            op0=mybir.AluOpType.mult)
                        nc.tensor.matmul(acc[:], lhsT=se[:], rhs=msg[:],
                                         start=(g == 0), stop=(g == g_n - 1))
                        j += 1
                    # finalize node tile
                    r = fin.tile([P, 2], f32, tag="r")
                    dpair = acc[:].rearrange("p (a b) -> p a b", b=66)[:, :, 64:65]
                    nc.vector.reciprocal(r[:], dpair)
                    t0 = fin.tile([P, 64], f32, tag="t0")
                    nc.vector.tensor_scalar(out=t0[:], in0=acc[:, 0:64],
                                            scalar1=r[:, 0:1], scalar2=None,
                                            op0=mybir.AluOpType.mult)
                    tsum = fin.tile([P, 64], f32, tag="tsum")
                    nc.vector.scalar_tensor_tensor(
                        out=tsum[:], in0=acc[:, 66:130], scalar=r[:, 1:2],
                        in1=t0[:], op0=mybir.AluOpType.mult,
                        op1=mybir.AluOpType.add)
                    b2 = fin.tile([P, 64], f32, tag="b2")
                    nc.vector.tensor_tensor(out=b2[:], in0=tsum[:], in1=biasr_sb[:],
                                            op=mybir.AluOpType.add)
                    q = fin.tile([P, 64], f32, tag="q")
                    nc.scalar.activation(q[:], b2[:],
                                         mybir.ActivationFunctionType.Relu)
                    mn = fin.tile([P, 64], f32, tag="mn")
                    nc.vector.tensor_tensor(out=mn[:], in0=b2[:], in1=q[:],
                                            op=mybir.AluOpType.subtract)
                    e = fin.tile([P, 64], f32, tag="e")
                    nc.scalar.activation(e[:], mn[:],
                                         mybir.ActivationFunctionType.Exp)
                    o = fin.tile([P, 64], f32, tag="o")
                    nc.vector.scalar_tensor_tensor(
                        out=o[:], in0=e[:], scalar=-1.0, in1=q[:],
                        op0=mybir.AluOpType.add, op1=mybir.AluOpType.add)
                    nc.sync.dma_start(OUT[t_i * P:(t_i + 1) * P, :], o[:])
                col0 += kq
    if legalize:
        _split_multi_waits(nc)
    return nc


def _split_multi_waits(nc, max_waits=1):
    """Walrus codegen encodes at most one sync wait on most ISA structs and
    does not reliably spill extras; hoist surplus waits onto same-engine NoOps
    placed immediately before the instruction (engine streams are per-engine
    in-order, so semantics are preserved)."""
    n = 0
    for f in nc.m.functions:
        for b in f.blocks:
            new = []
            for inst in b.instructions:
                si = inst.sync_info
                waits = list(si.on_wait) if si is not None and si.on_wait else []
                if len(waits) > max_waits:
                    extras, keep = waits[:-max_waits], waits[-max_waits:]
                    for w in extras:
                        n += 1
                        new.append(mybir.InstNoOp(
                            name=f"waitnop-{n}",
                            engine=inst.engine,
                            sync_info=mybir.SyncInfo(on_wait=[w], on_update=[]),
                        ))
                    si.on_wait = keep
                new.append(inst)
            b.instructions = new
    return n


def _host_consts(W, att_src, att_dst, bias):
    Fin, H, C = W.shape
    wp = np.zeros((65, 136), np.float32)
    # xl stored pre-scaled by 0.5 (head-mean factor); denom cols unaffected,
    # so msg/denom comes out as 0.5 * sum(alpha*xl) directly.
    wp[:64, 0:64] = 0.5 * W[:, 0, :]
    wp[:64, 66:130] = 0.5 * W[:, 1, :]
    wp[:64, 132] = W[:, 0, :] @ att_src[0]
    wp[:64, 133] = W[:, 1, :] @ att_src[1]
    wp[:64, 134] = W[:, 0, :] @ att_dst[0]
    wp[:64, 135] = W[:, 1, :] @ att_dst[1]
    # ones-row -> constant 1.0 in the denominator columns of every table row
    wp[64, 64] = 1.0
    wp[64, 130] = 1.0
    iota = np.tile(np.arange(P, dtype=np.float16)[None, :], (P, 1))
    biasr = np.tile(np.asarray(bias, np.float32)[None, :], (P, 1))
    dconst = np.zeros((P, 136), np.float16)
    dconst[:, 64] = 1.0
    dconst[:, 130] = 1.0
    dconst[:, 132] = DUMMY_AS
    dconst[:, 133] = DUMMY_AS
    return wp, iota, biasr, dconst


def gat_run(h, src, dst, W, att_src, att_dst, bias, q_tiles=4, run_kwargs=None):
    """Shared implementation: returns (output [B,N,T,C] f32, BassKernelResults)."""
    h = np.asarray(h, np.float32)
    Bb, N, Tt, Fin = h.shape
    n_pad = math.ceil(N / P) * P
    prep = _prep_graph(np.asarray(src), np.asarray(dst), N, n_pad)
    nc = _build_nc(n_pad, prep["g_per_tile"], q_tiles)
    wp, iota, biasr, dconst = _host_consts(
        np.asarray(W, np.float32), np.asarray(att_src, np.float32),
        np.asarray(att_dst, np.float32), np.asarray(bias, np.float32))

    n_cores = Bb * Tt
    in_maps = []
    for t in range(n_cores):
        xtt = np.zeros((65, n_pad), np.float32)
        xtt[:64, :N] = h[0, :, t, :].T
        xtt[64, :] = 1.0
        in_maps.append({
            "XTT": np.ascontiguousarray(xtt),
            "WPLUS": wp, "IOTA": iota, "BIASR": biasr, "DCONST": dconst,
            "SRCI": prep["src_plane"], "DSTI": prep["dst_plane"],
            "DREL": prep["drel_plane"],
        })
    res = run_bass_kernel_spmd(nc, in_maps, core_ids=list(range(n_cores)),
                               **(run_kwargs or {}))
    outs = [res.results[t]["OUT"][:N] for t in range(n_cores)]
    out = np.stack(outs, axis=0).reshape(Bb, Tt, N, 64).transpose(0, 2, 1, 3)
    return np.ascontiguousarray(out.astype(np.float32)), res


def kernel(h, src, dst, W, att_src, att_dst, bias):
    out, _ = gat_run(h, src, dst, W, att_src, att_dst, bias)
    return out

